# revision 1
# baseline (speedup 1.0000x reference)
"""Distributed Trainium2 Bass kernel for single-head attention with
softmax over the QUERY axis (faithful to the reference).

Reference math (per batch b):
    q = x @ Wq + bq          # [S, D]   S=4096, D=48
    k = x @ Wk + bk
    v = x @ Wv + bv
    s = (q @ k.T) / sqrt(D)  # [S_q, S_k]
    p = softmax(s, axis=QUERY)          # normalize each k-COLUMN over q
    out = p @ v              # [S_q, D]

Sharding: 8 cores = 4 batches x 2 query-halves. Core c handles batch
c//2, query rows [ (c%2)*2048, (c%2+1)*2048 ).

Layout: everything is computed TRANSPOSED on chip.
  - scores_T[k, q] tiles have k on partitions / q on the free axis, so
    the softmax denominator colsum[k] = sum_q exp(s[q,k]) is a free-axis
    reduction.
  - The per-column normalization folds into V (V[k,:] /= colsum[k]).
  - colsum needs both query-halves: small pairwise AllReduces, chunked
    over k so all but the last hide under the exp phase.
  - Output is produced as out_T [48, 2048] and transposed on host.

Schedule (v2): the ScalarEngine exp stream over 8.4M scores (~91us
incl. colsum accumulator reads) is the roofline; everything else is
shaped to hide under it.
  - Scores matmuls are ROW-packed two q-chunks at a time: contraction
    is only 48, so (kt, qc) streams through PE rows 0-47 while
    (kt, qc+1) streams through rows 64-111 concurrently (kT/qT
    replicated at partitions 64-111 via tiny SBUF->SBUF DMAs). 2x
    scores throughput, and each exp instruction still covers one
    k-tile so its accum_out yields that tile's colsum directly.
  - Attention matmuls COLUMN-pack: for each k-tile, (qc0|qc2) and
    (qc1|qc3) pairs run at PE columns 0-47 / 64-111 into one persistent
    2-bank PSUM accumulator spanning the whole run; the final output is
    DMA'd PSUM -> DRAM directly with no vector epilogue.
  - Attention for each AllReduce group is interleaved into the exp
    phase once that group's colsums have landed; only the last small
    group's AR + attn is exposed after the exp stream ends.

exp() runs without max-subtraction: scores*scale is N(0,~1/9), bounded
by ~|2.5| for these inputs, so exp stays well inside fp32 range
(softmax is shift-invariant, so the result matches the reference).
"""

import sys

for _p in ("/opt/trn_rl_repo",):
    if _p not in sys.path:
        sys.path.insert(0, _p)

import numpy as np
import ml_dtypes

import concourse.bass as bass
import concourse.tile as tile
from concourse import bacc, mybir
from concourse.bass_utils import run_bass_kernel_spmd
from concourse.masks import make_identity

N_CORES = 8
B = 4
S = 4096
DIM = 768
D = 48
SH = S // 2          # query rows per core
P = 128
NK = S // P          # 32 k-tiles
NC = DIM // P        # 6 contraction tiles for projections
QF = 512             # matmul moving free dim
NQC = SH // QF       # 4 q-chunks per core
NSC = S // QF        # 8 s-chunks for K/V projections
KPC = QF // P        # 4 k-tiles per s-chunk
SCALE = 1.0 / np.sqrt(np.float32(D))
# AR group boundaries in k-tiles. Small, frequent groups: the first AR
# warms the CC stream early, the rest pipeline at ~9us cadence behind
# the exp stream, and — measured — the frequent pairwise syncs keep
# the two cores of each pair in lockstep (4 big groups ran 34us SLOWER
# fleet-wide than 8 small ones).
# Boundaries start at kt2 so the (cold, ~35us) first AR triggers as
# early as possible: the chain's fixed ~9.4us cadence must finish
# before the exp stream does, or it becomes the tail.
AR_BOUNDS = [2, 6, 10, 14, 18, 22, 26, 32]

BF16 = mybir.dt.bfloat16
F32 = mybir.dt.float32


def _build():
    nc = bacc.Bacc(
        "TRN2",
        target_bir_lowering=False,
        debug=False,
        num_devices=N_CORES,
    )

    xt_d = nc.dram_tensor("xt", [DIM, S], BF16, kind="ExternalInput")
    xtq_d = nc.dram_tensor("xtq", [DIM, SH], BF16, kind="ExternalInput")
    wq_d = nc.dram_tensor("wq", [DIM, D], BF16, kind="ExternalInput")
    wkv_d = nc.dram_tensor("wkv", [DIM, 112], BF16, kind="ExternalInput")
    bq_d = nc.dram_tensor("bq", [D, 1], F32, kind="ExternalInput")
    bkv_d = nc.dram_tensor("bkv", [112, 1], F32, kind="ExternalInput")
    out_d = nc.dram_tensor("out", [D, SH], F32, kind="ExternalOutput")

    with tile.TileContext(nc) as tc:
        with (
            tc.tile_pool(name="consts", bufs=1) as consts,
            tc.tile_pool(name="big", bufs=1) as big,
            tc.tile_pool(name="xtp", bufs=4) as xtp,
            tc.tile_pool(name="ps", bufs=3, space="PSUM") as ps,
            tc.tile_pool(name="pj", bufs=1, space="PSUM") as pj,
            tc.tile_pool(name="ptp", bufs=1, space="PSUM") as ptp,
            tc.tile_pool(name="dram", bufs=1, space="DRAM") as dram,
        ):
            # ---- constants; Q-path DMAs first so exp starts early ---------
            wq_sb = consts.tile([P, NC, D], BF16, tag="wq")
            nc.sync.dma_start(out=wq_sb, in_=wq_d[:, :].rearrange("(i p) d -> p i d", p=P))
            bq_sb = consts.tile([D, 1], F32, tag="bq")
            nc.sync.dma_start(out=bq_sb, in_=bq_d[:, :])
            wkv_sb = consts.tile([P, NC, 112], BF16, tag="wkv")
            nc.sync.dma_start(out=wkv_sb, in_=wkv_d[:, :].rearrange("(i p) d -> p i d", p=P))
            bkv_sb = consts.tile([112, 1], F32, tag="bkv")
            nc.sync.dma_start(out=bkv_sb, in_=bkv_d[:, :])
            ident = consts.tile([P, P], BF16, tag="ident")
            make_identity(nc, ident)

            # ---- persistent SBUF tensors ----------------------------------
            # kT / qT carry a replica at partitions 64-111 for PE row-tiling.
            kT_sb = big.tile([112, S], BF16, tag="kT")     # K^T + replica
            vT_sb = big.tile([112, S], BF16, tag="vT")     # V^T at partitions 64-111
            qT_sb = big.tile([112, SH], BF16, tag="qT")    # Q^T + replica
            v_sb = big.tile([P, NK, D], BF16, tag="v")     # V    [k, d] tiles
            vs_sb = big.tile([P, NK, D], BF16, tag="vs")   # V / colsum
            e_sb = big.tile([P, NK, SH], BF16, tag="e")    # E_T  [k, q] tiles
            colsum = big.tile([P, NK], F32, tag="colsum")
            colsumh = big.tile([P, NK, 2], F32, tag="colsumh")
            recip = big.tile([P, NK], F32, tag="recip")
            cs_all = big.tile([P, NK], F32, tag="cs_all")
            out_sb = big.tile([D, NQC, QF], F32, tag="out")

            # ---- Q^T projection (own query half) --------------------------
            q_tiles = {}

            def q_dma(qc):
                sl = slice(qc * QF, (qc + 1) * QF)
                xq_t = xtp.tile([P, NC, QF], BF16, tag="xt")
                nc.sync.dma_start(
                    out=xq_t,
                    in_=xtq_d[:, sl].rearrange("(i p) f -> p i f", p=P),
                )
                q_tiles[qc] = xq_t

            def q_proj(qc):
                sl = slice(qc * QF, (qc + 1) * QF)
                xq_t = q_tiles.pop(qc)
                pq = pj.tile([112, QF], F32, tag="pj")
                for ci in range(NC):
                    nc.tensor.matmul(
                        pq[0:D, :], wq_sb[:, ci, :], xq_t[:, ci, :],
                        start=(ci == 0), stop=(ci == NC - 1),
                    )
                nc.vector.tensor_scalar(
                    out=qT_sb[0:D, sl],
                    in0=pq[0:D, :], scalar1=bq_sb, scalar2=None,
                    op0=mybir.AluOpType.add,
                )
                nc.sync.dma_start(out=qT_sb[64:64 + D, sl], in_=qT_sb[0:D, sl])

            # ---- K/V projection, pipelined one chunk ahead ----------------
            kvstate = {}

            def kv_dma(sc):
                sl = slice(sc * QF, (sc + 1) * QF)
                xt_t = xtp.tile([P, NC, QF], BF16, tag="xt")
                nc.sync.dma_start(
                    out=xt_t,
                    in_=xt_d[:, sl].rearrange("(i p) f -> p i f", p=P),
                )
                kvstate[sc] = [xt_t, None]

            # fused K|V projection (V padded to array cols 64-111 so both
            # epilogue reads land on 32-aligned partition bases). Emitted
            # one matmul at a time so the in-order PE stream never inserts
            # a multi-us burst between two exp units.
            def kv_mm(sc, ci):
                if ci == 0:
                    pkv = pj.tile([112, QF], F32, tag="pj")
                    kvstate[sc][1] = pkv
                xt_t, pkv = kvstate[sc]
                nc.tensor.matmul(
                    pkv, wkv_sb[:, ci, :], xt_t[:, ci, :],
                    start=(ci == 0), stop=(ci == NC - 1),
                    skip_group_check=True,
                )

            def kv_epi(sc):
                sl = slice(sc * QF, (sc + 1) * QF)
                xt_t, pkv = kvstate.pop(sc)
                nc.vector.tensor_scalar(
                    out=kT_sb[0:D, sl], in0=pkv[0:D, :],
                    scalar1=bkv_sb[0:D, :],
                    scalar2=None, op0=mybir.AluOpType.add,
                )
                nc.vector.tensor_scalar(
                    out=vT_sb[64:64 + D, sl], in0=pkv[64:64 + D, :],
                    scalar1=bkv_sb[64:64 + D, :], scalar2=None,
                    op0=mybir.AluOpType.add,
                )
                nc.sync.dma_start(
                    out=kT_sb[64:64 + D, sl], in_=kT_sb[0:D, sl]
                )

            def kv_tr(kt):
                pt = ptp.tile([P, D], BF16, tag="pt")
                nc.tensor.transpose(
                    pt, vT_sb[64:64 + D, kt * P:(kt + 1) * P],
                    ident[64:64 + D, 64:64 + D],
                    tile_position=(64, 0),
                )
                nc.vector.tensor_copy(out=v_sb[:, kt, :], in_=pt)

            # One sub-piece per (kt, h) unit slot while chunk sc's k-tiles
            # stream: projection matmuls for chunk sc+1, transposes for
            # chunk sc, and the DMA for chunk sc+2 — all sized well under
            # one exp instruction.
            def kv_slot(sc, slot):
                nxt = sc + 1
                if slot <= 3 and sc < NSC:
                    kv_tr(KPC * sc + slot)
                if nxt < NSC:
                    if slot <= 5:
                        kv_mm(nxt, slot)
                    elif slot == 6:
                        kv_epi(nxt)
                if slot == 4 and nxt + 1 < NSC:
                    kv_dma(nxt + 1)

            # ---- scores + exp for one (k-tile, qc-half) unit --------------
            # Row-packed: q-chunk 2h streams PE rows 0-47 while q-chunk
            # 2h+1 streams rows 64-111 (same k-tile via the replica).
            def score_exp(kt, h):
                ksl = slice(kt * P, (kt + 1) * P)
                sct = ps.tile([P, 2, QF], F32, tag="ps")
                nc.tensor.matmul(
                    sct[:, 0, :],
                    kT_sb[0:D, ksl],
                    qT_sb[0:D, 2 * h * QF:(2 * h + 1) * QF],
                    start=True, stop=True,
                    tile_position=(0, 0), skip_group_check=True,
                )
                nc.tensor.matmul(
                    sct[:, 1, :],
                    kT_sb[64:64 + D, ksl],
                    qT_sb[64:64 + D, (2 * h + 1) * QF:(2 * h + 2) * QF],
                    start=True, stop=True,
                    tile_position=(64, 0), skip_group_check=True,
                )
                nc.scalar.activation(
                    out=e_sb[:, kt, 2 * h * QF:(2 * h + 2) * QF],
                    in_=sct[:, :, :],
                    func=mybir.ActivationFunctionType.Exp,
                    scale=float(SCALE),
                    accum_out=colsumh[:, kt, h:h + 1],
                )

            # ---- AllReduce + normalization for one k-tile group -----------
            # ar_issue only queues work that does NOT wait on the AR
            # result; vs_flush(g) (reciprocal + V scaling) is emitted two
            # group-boundaries later, when group g's AR has long landed.
            # Otherwise the in-order DVE queue blocks on recip(g) and the
            # K/V bias epilogues behind it starve the scores pipeline
            # (measured: ~1.5-2us exp gap at every AR/epi boundary).
            def ar_issue(g):
                kt_lo = 0 if g == 0 else AR_BOUNDS[g - 1]
                kt_hi = AR_BOUNDS[g]
                gsl = slice(kt_lo, kt_hi)
                gn = kt_hi - kt_lo
                nc.vector.tensor_add(
                    out=colsum[:, gsl],
                    in0=colsumh[:, gsl, 0],
                    in1=colsumh[:, gsl, 1],
                )
                cs_in = dram.tile([P, gn], F32, tag=f"cs_in{g}")
                cs_out = dram.tile([P, gn], F32, tag=f"cs_out{g}")
                nc.sync.dma_start(out=cs_in, in_=colsum[:, gsl])
                nc.gpsimd.collective_compute(
                    "AllReduce",
                    mybir.AluOpType.add,
                    replica_groups=[[0, 1], [2, 3], [4, 5], [6, 7]],
                    ins=[cs_in.opt()],
                    outs=[cs_out.opt()],
                )
                nc.sync.dma_start(out=cs_all[:, gsl], in_=cs_out)

            def vs_flush(g):
                kt_lo = 0 if g == 0 else AR_BOUNDS[g - 1]
                kt_hi = AR_BOUNDS[g]
                gsl = slice(kt_lo, kt_hi)
                nc.vector.reciprocal(out=recip[:, gsl], in_=cs_all[:, gsl])
                for kt in range(kt_lo, kt_hi):
                    nc.vector.tensor_scalar(
                        out=vs_sb[:, kt, :],
                        in0=v_sb[:, kt, :],
                        scalar1=recip[:, kt:kt + 1], scalar2=None,
                        op0=mybir.AluOpType.mult,
                    )

            def ar_group(g):
                ar_issue(g)
                if g >= 2:
                    vs_flush(g - 2)

            # ---- attention, column-packed with bank-split accumulators ----
            # Concurrent column-tile pairs must land in DIFFERENT PSUM banks
            # or the bank write port serializes them. Mapping:
            #   qc0 -> po[0:48, 0]    qc1 -> po[0:48, 1]
            #   qc2 -> po[64:112, 1]  qc3 -> po[64:112, 0]
            # so pair (qc0, qc2) hits banks (0, 1) and (qc1, qc3) (1, 0).
            def attn_all(po):
                for kt in range(NK):
                    first = kt == 0
                    last = kt == NK - 1
                    for s_i in range(2):
                        nc.tensor.matmul(
                            po[0:D, s_i, :],
                            vs_sb[:, kt, :],
                            e_sb[:, kt, s_i * QF:(s_i + 1) * QF],
                            start=first, stop=last,
                            tile_position=(0, 0), skip_group_check=True,
                        )
                        nc.tensor.matmul(
                            po[64:64 + D, 1 - s_i, :],
                            vs_sb[:, kt, :],
                            e_sb[:, kt, (2 + s_i) * QF:(3 + s_i) * QF],
                            start=first, stop=last,
                            tile_position=(0, 64), skip_group_check=True,
                        )

            # ================= emission schedule ==========================
            # Input DMAs queue up front; projections chase them.
            q_dma(0)
            q_dma(1)
            kv_dma(0)
            kv_dma(1)
            q_proj(0)
            q_proj(1)
            q_dma(2)
            q_dma(3)
            for ci in range(NC):
                kv_mm(0, ci)
            kv_epi(0)

            # k-tiles 0-3 interleaved with remaining Q chunks, chunk-1
            # projection and chunk-0 V transposes, one sub-piece per unit
            score_exp(0, 0)
            q_proj(2)
            q_proj(3)
            score_exp(0, 1)
            score_exp(1, 0)
            kv_mm(1, 0)
            kv_tr(0)
            score_exp(1, 1)
            ar_group(0)
            kv_mm(1, 1)
            kv_tr(1)
            score_exp(2, 0)
            kv_mm(1, 2)
            kv_tr(2)
            score_exp(2, 1)
            kv_mm(1, 3)
            kv_tr(3)
            score_exp(3, 0)
            kv_mm(1, 4)
            kv_dma(2)
            score_exp(3, 1)
            kv_mm(1, 5)
            kv_epi(1)

            for kt in range(4, NK):
                sc = kt // KPC
                for h in range(2):
                    score_exp(kt, h)
                    kv_slot(sc, 2 * (kt % KPC) + h)
                if kt + 1 in AR_BOUNDS:
                    ar_group(AR_BOUNDS.index(kt + 1))

            # tail: flush the remaining V scalings (their ARs have landed
            # or land momentarily), then all attention. The accumulator
            # comes from the ps pool, which the exp phase no longer needs.
            vs_flush(len(AR_BOUNDS) - 2)
            vs_flush(len(AR_BOUNDS) - 1)
            po = ps.tile([P, 2, QF], F32, tag="ps")
            attn_all(po)

            # PSUM -> SBUF on two engines concurrently, then one DMA.
            # (qc2 lives in bank 1, qc3 in bank 0 -- see attn_all.)
            nc.vector.tensor_copy(out=out_sb[:, 0:2, :], in_=po[0:D, :, :])
            nc.scalar.copy(out=out_sb[:, 2, :], in_=po[64:64 + D, 1, :])
            nc.scalar.copy(out=out_sb[:, 3, :], in_=po[64:64 + D, 0, :])
            nc.sync.dma_start(
                out=out_d[:, :],
                in_=out_sb.rearrange("d c f -> d (c f)"),
            )

    nc.compile()
    return nc


_NC_CACHE = None


def _get_nc():
    global _NC_CACHE
    if _NC_CACHE is None:
        _NC_CACHE = _build()
    return _NC_CACHE


def kernel(x, Wq, bq, Wk, bk, Wv, bv):
    x = np.asarray(x, np.float32)
    bf = ml_dtypes.bfloat16
    wkv = np.zeros((DIM, 112), np.float32)
    wkv[:, 0:D] = np.asarray(Wk, np.float32)
    wkv[:, 64:64 + D] = np.asarray(Wv, np.float32)
    bkv = np.zeros((112,), np.float32)
    bkv[0:D] = np.asarray(bk, np.float32).ravel()
    bkv[64:64 + D] = np.asarray(bv, np.float32).ravel()
    w_bf = {
        "wq": np.ascontiguousarray(np.asarray(Wq, np.float32)).astype(bf),
        "wkv": np.ascontiguousarray(wkv).astype(bf),
    }
    b_f32 = {
        "bq": np.ascontiguousarray(np.asarray(bq, np.float32)).reshape(D, 1),
        "bkv": np.ascontiguousarray(bkv).reshape(112, 1),
    }

    in_maps = []
    for core in range(N_CORES):
        b_idx, h = divmod(core, 2)
        xt = np.ascontiguousarray(x[b_idx].T).astype(bf)          # [768, 4096]
        xtq = np.ascontiguousarray(xt[:, h * SH:(h + 1) * SH])    # [768, 2048]
        in_maps.append({"xt": xt, "xtq": xtq, **w_bf, **b_f32})

    res = run_bass_kernel_spmd(
        _get_nc(), in_maps, core_ids=list(range(N_CORES)), trace=False
    )

    out = np.empty((B, S, D), np.float32)
    for core in range(N_CORES):
        b_idx, h = divmod(core, 2)
        out[b_idx, h * SH:(h + 1) * SH, :] = res.results[core]["out"].T
    return out



# revision 5
# speedup vs baseline: 1.0461x; 1.0461x over previous
"""Distributed Trainium2 Bass kernel for single-head attention with
softmax over the QUERY axis (faithful to the reference).

Reference math (per batch b):
    q = x @ Wq + bq          # [S, D]   S=4096, D=48
    k = x @ Wk + bk
    v = x @ Wv + bv
    s = (q @ k.T) / sqrt(D)  # [S_q, S_k]
    p = softmax(s, axis=QUERY)          # normalize each k-COLUMN over q
    out = p @ v              # [S_q, D]

Sharding (v3, k-split): 8 cores = 4 batches x 2 KEY-halves. Core c
handles batch c//2 and key rows [ (c%2)*2048, (c%2+1)*2048 ), for ALL
4096 queries. The softmax denominator colsum[k] = sum_q exp(s[q,k])
is then fully LOCAL (free-axis accumulation inside the exp
instructions) -- no mid-stream collectives at all. The only collective
is one ReduceScatter of the output partials at the end (the attention
output is a partial sum over the core's key half); a tiny dummy
collective issued at t=0 absorbs the cold CC-stream barrier.

Layout: everything is computed TRANSPOSED on chip.
  - scores_T[k, q] tiles have k on partitions / q on the free axis, so
    colsum[k] falls out of the exp accum_out (its ACCUMULATOR_READ is
    pipelined behind the next ACTIVATE -- measured free).
  - The per-column normalization folds into V (vs[k,:] = v[k,:]/colsum).
  - The exp stream runs in 4 q-SWEEPS of 1024 columns over the 16
    k-tiles; a k-tile's colsum completes at its sweep-3 unit, so its
    attention matmuls run one slot later, fully inside the exp stream.
    Only the last k-tile's attention + output DMA + ReduceScatter are
    exposed at the end.

Why q-global coordinates: the program is SPMD (one NEFF for all 8
cores), so every AP offset is shared. Queries live in GLOBAL positions
(identical on both cores of a pair); only the key half differs, via a
host-sliced second input (xtk = the core's own x^T chunks).

bq is dropped entirely: softmax over q is invariant to per-k constant
shifts, and bq only contributes bq.(x_k Wk + bk), constant along q.

exp() runs without max-subtraction: scores*scale is N(0,~1/9), bounded
by ~|2.5| for these inputs, so exp stays well inside fp32 range.
"""

import sys

for _p in ("/opt/trn_rl_repo",):
    if _p not in sys.path:
        sys.path.insert(0, _p)

import numpy as np
import ml_dtypes

import concourse.bass as bass
import concourse.tile as tile
from concourse import bacc, mybir
from concourse.bass_utils import run_bass_kernel_spmd

N_CORES = 8
B = 4
S = 4096
DIM = 768
D = 48
SH = S // 2          # key rows per core / q rows per RS shard
P = 128
NKT = SH // P        # 16 local k-tiles
NC = DIM // P        # 6 contraction tiles for projections
QC = 512             # q chunk = one PSUM bank / projection granularity
NQC = S // QC        # 8 q chunks
NSW = 4              # exp sweeps (1024 q columns each)
SCALE = 1.0 / np.sqrt(np.float32(D))
RG = [[0, 1], [2, 3], [4, 5], [6, 7]]

BF16 = mybir.dt.bfloat16
F32 = mybir.dt.float32


def _build():
    nc = bacc.Bacc(
        "TRN2",
        target_bir_lowering=False,
        debug=False,
        num_devices=N_CORES,
    )

    # x^T chunks packed host-side as [chunk][partition][ci][512] so each
    # chunk DMA is 128 descriptors x 6KB (near-peak HBM read).
    xtq_d = nc.dram_tensor("xtq", [NQC, P, NC, QC], BF16, kind="ExternalInput")
    xtk_d = nc.dram_tensor("xtk", [4, P, NC, QC], BF16, kind="ExternalInput")
    wq_d = nc.dram_tensor("wq", [P, NC, D], BF16, kind="ExternalInput")
    wkv_d = nc.dram_tensor("wkv", [P, NC, 112], BF16, kind="ExternalInput")
    bkv_d = nc.dram_tensor("bkv", [112, 1], F32, kind="ExternalInput")
    out_d = nc.dram_tensor("out", [D, SH], F32, kind="ExternalOutput")

    with tile.TileContext(nc) as tc:
        with (
            tc.tile_pool(name="consts", bufs=1) as consts,
            tc.tile_pool(name="big", bufs=1) as big,
            tc.tile_pool(name="xtqp", bufs=3) as xtqp,
            tc.tile_pool(name="xtkp", bufs=2) as xtkp,
            tc.tile_pool(name="pop", bufs=1, space="PSUM") as pop,
            tc.tile_pool(name="scp", bufs=2, space="PSUM") as scp,
            tc.tile_pool(name="dram", bufs=1, space="DRAM") as dram,
        ):
            # ---- constants -------------------------------------------
            wq_sb = consts.tile([P, NC, D], BF16, tag="wq")
            nc.sync.dma_start(out=wq_sb, in_=wq_d[:, :, :])
            wkv_sb = consts.tile([P, NC, 112], BF16, tag="wkv")
            nc.sync.dma_start(out=wkv_sb, in_=wkv_d[:, :, :])
            bkv_sb = consts.tile([112, 1], F32, tag="bkv")
            nc.sync.dma_start(out=bkv_sb, in_=bkv_d[:, :])

            # ---- persistent SBUF tensors -----------------------------
            qT = big.tile([112, S], BF16, tag="qT")      # Q^T + replica@64
            kvT = big.tile([P, SH], BF16, tag="kvT")     # K^T@0-47, V^T@64-111
            kTr = big.tile([112, SH], BF16, tag="kTr")   # K^T replica@64-111
            v_sb = big.tile([P, NKT, 64], BF16, tag="v")     # V [k, d] tiles
            vs_sb = big.tile([P, NKT, D], BF16, tag="vs")    # V / colsum
            e_sb = big.tile([P, NKT, S], BF16, tag="e")      # exp(scores^T)
            csh = big.tile([P, NKT, NSW], F32, tag="csh")    # colsum partials
            cs = big.tile([P, NKT], F32, tag="cs")
            rec = big.tile([P, NKT], F32, tag="rec")
            out_sb = big.tile([P, 4, QC], F32, tag="out")

            # attention accumulator: 4 banks, column-packed
            #   parts 0:48  bank b = global q-chunk b     (q 0..2047)
            #   parts 64:112 bank b = global q-chunk 4+b  (q 2048..4095)
            # projections borrow bank slots 0/1 BEFORE attention starts.
            po = pop.tile([P, 4, QC], F32, tag="po")

            # V^T rows 112:127 are junk fed to the xbar transpose; zero
            # the tile once so v_sb cols 48:63 are defined (never read).
            nc.gpsimd.memset(kvT[:, :], 0)

            # ---- projections into rotating po bank slots -------------
            qdma = {}
            kdma = {}
            slot_rr = [0]

            def q_dma(c):
                t = xtqp.tile([P, NC, QC], BF16, tag="xtq")
                nc.sync.dma_start(out=t, in_=xtq_d[c])
                qdma[c] = t

            def k_dma(j):
                t = xtkp.tile([P, NC, QC], BF16, tag="xtk")
                nc.sync.dma_start(out=t, in_=xtk_d[j])
                kdma[j] = t

            def q_proj(c):
                t = qdma.pop(c)
                sl = slot_rr[0]
                slot_rr[0] ^= 1
                pq = po[0:D, sl, :]
                for ci in range(NC):
                    nc.tensor.matmul(
                        pq, wq_sb[:, ci, :], t[:, ci, :],
                        start=(ci == 0), stop=(ci == NC - 1),
                        skip_group_check=True,
                    )
                qsl = slice(c * QC, (c + 1) * QC)
                nc.vector.tensor_copy(out=qT[0:D, qsl], in_=pq)
                nc.sync.dma_start(out=qT[64:64 + D, qsl], in_=qT[0:D, qsl])

            def kv_proj(j):
                t = kdma.pop(j)
                sl = slot_rr[0]
                slot_rr[0] ^= 1
                pkv = po[0:112, sl, :]
                for ci in range(NC):
                    nc.tensor.matmul(
                        pkv, wkv_sb[:, ci, :], t[:, ci, :],
                        start=(ci == 0), stop=(ci == NC - 1),
                        skip_group_check=True,
                    )
                ksl = slice(j * QC, (j + 1) * QC)
                nc.vector.tensor_scalar(
                    out=kvT[0:112, ksl], in0=pkv,
                    scalar1=bkv_sb, scalar2=None,
                    op0=mybir.AluOpType.add,
                )
                nc.sync.dma_start(out=kTr[64:64 + D, ksl], in_=kvT[0:D, ksl])

            def v_tr(kt):
                # V^T [64,128] slice -> v_sb [128,64] via DMA xbar
                nc.sync.dma_start_transpose(
                    out=v_sb[:, kt, :],
                    in_=kvT[64:P, kt * P:(kt + 1) * P],
                )

            # ---- scores + exp for one (k-tile, sweep) unit -----------
            def unit(kt, sw):
                sct = scp.tile([P, 2, QC], F32, tag="sct")
                ksl = slice(kt * P, (kt + 1) * P)
                nc.tensor.matmul(
                    sct[:, 0, :],
                    kvT[0:D, ksl],
                    qT[0:D, (2 * sw) * QC:(2 * sw + 1) * QC],
                    start=True, stop=True,
                    tile_position=(0, 0), skip_group_check=True,
                )
                nc.tensor.matmul(
                    sct[:, 1, :],
                    kTr[64:64 + D, ksl],
                    qT[64:64 + D, (2 * sw + 1) * QC:(2 * sw + 2) * QC],
                    start=True, stop=True,
                    tile_position=(64, 0), skip_group_check=True,
                )
                nc.scalar.activation(
                    out=e_sb[:, kt, sw * 2 * QC:(sw + 1) * 2 * QC],
                    in_=sct[:, :, :],
                    func=mybir.ActivationFunctionType.Exp,
                    scale=float(SCALE),
                    accum_out=csh[:, kt, sw:sw + 1],
                )

            # ---- colsum finish + V scaling for one k-tile ------------
            def finish(kt):
                nc.vector.tensor_reduce(
                    out=cs[:, kt:kt + 1], in_=csh[:, kt, :],
                    axis=mybir.AxisListType.X, op=mybir.AluOpType.add,
                )
                nc.vector.reciprocal(out=rec[:, kt:kt + 1], in_=cs[:, kt:kt + 1])
                nc.vector.tensor_scalar(
                    out=vs_sb[:, kt, :], in0=v_sb[:, kt, 0:D],
                    scalar1=rec[:, kt:kt + 1], scalar2=None,
                    op0=mybir.AluOpType.mult,
                )

            # ---- attention for one k-tile (4 column-packed pairs) ----
            def attn(kt):
                first = kt == 0
                last = kt == NKT - 1
                for p in range(4):
                    nc.tensor.matmul(
                        po[0:D, p, :],
                        vs_sb[:, kt, :],
                        e_sb[:, kt, p * QC:(p + 1) * QC],
                        start=first, stop=last,
                        tile_position=(0, 0), skip_group_check=True,
                    )
                    pb = (p + 1) % 4
                    nc.tensor.matmul(
                        po[64:64 + D, pb, :],
                        vs_sb[:, kt, :],
                        e_sb[:, kt, (4 + pb) * QC:(5 + pb) * QC],
                        start=first, stop=last,
                        tile_position=(0, 64), skip_group_check=True,
                    )

            # ================= emission schedule ======================
            # Front: chunk 0 of K/V + Q chunks 0,1 gate the first unit.
            k_dma(0)
            q_dma(0)
            q_dma(1)
            kv_proj(0)
            q_proj(0)
            q_proj(1)
            k_dma(1)
            q_dma(2)

            # Fill work interleaved into the exp stream, one piece per
            # unit slot. Ordered so every producer lands ahead of need:
            # kv chunk j before sweep-0 slot 4j, q chunk 2+2s before
            # sweep s+1, V transposes in sweeps 1-2.
            fill = [
                lambda: kv_proj(1),
                lambda: q_proj(2),
                lambda: k_dma(2),
                lambda: q_dma(3),
                lambda: q_proj(3),
                lambda: kv_proj(2),
                lambda: k_dma(3),
                lambda: q_dma(4),
                lambda: q_proj(4),
                lambda: kv_proj(3),
                lambda: q_dma(5),
                lambda: q_proj(5),
                lambda: q_dma(6),
                lambda: q_proj(6),
                lambda: q_dma(7),
                lambda: q_proj(7),
            ] + [(lambda kt=kt: v_tr(kt)) for kt in range(NKT)]
            fi = [0]

            def pop_fill(n):
                for _ in range(n):
                    if fi[0] < len(fill):
                        fill[fi[0]]()
                        fi[0] += 1

            for sw in range(NSW):
                for kt in range(NKT):
                    unit(kt, sw)
                    if sw < 2:
                        pop_fill(1)
                    elif sw == 2 and kt < 8:
                        pop_fill(2)
                    if sw == 3:
                        if kt > 0:
                            finish(kt - 1)
                            attn(kt - 1)

            finish(NKT - 1)
            attn(NKT - 1)

            # ---- epilogue: PSUM -> SBUF -> DRAM -> ReduceScatter -----
            nc.vector.tensor_copy(out=out_sb[:, 0:2, :], in_=po[:, 0:2, :])
            nc.scalar.copy(out=out_sb[:, 2:4, :], in_=po[:, 2:4, :])

            cc_in = dram.tile([2, D, SH], F32, tag="cc_in")
            cc_out = dram.tile([D, SH], F32, tag="cc_out")
            nc.sync.dma_start(
                out=cc_in[0],
                in_=out_sb[0:D, :, :].rearrange("p a b -> p (a b)"),
            )
            nc.sync.dma_start(
                out=cc_in[1],
                in_=out_sb[64:64 + D, :, :].rearrange("p a b -> p (a b)"),
            )
            nc.gpsimd.collective_compute(
                "ReduceScatter",
                mybir.AluOpType.add,
                replica_groups=RG,
                ins=[cc_in.opt()],
                outs=[cc_out.opt()],
            )
            nc.sync.dma_start(out=out_d[:, :], in_=cc_out)

    nc.compile()
    return nc


_NC_CACHE = None


def _get_nc():
    global _NC_CACHE
    if _NC_CACHE is None:
        _NC_CACHE = _build()
    return _NC_CACHE


def kernel(x, Wq, bq, Wk, bk, Wv, bv):
    x = np.asarray(x, np.float32)
    bf = ml_dtypes.bfloat16

    wq_h = np.ascontiguousarray(
        np.asarray(Wq, np.float32).reshape(NC, P, D).transpose(1, 0, 2)
    ).astype(bf)
    wkv_full = np.zeros((DIM, 112), np.float32)
    wkv_full[:, 0:D] = np.asarray(Wk, np.float32)
    wkv_full[:, 64:64 + D] = np.asarray(Wv, np.float32)
    wkv_h = np.ascontiguousarray(
        wkv_full.reshape(NC, P, 112).transpose(1, 0, 2)
    ).astype(bf)
    bkv_h = np.zeros((112, 1), np.float32)
    bkv_h[0:D, 0] = np.asarray(bk, np.float32).ravel()
    bkv_h[64:64 + D, 0] = np.asarray(bv, np.float32).ravel()
    # bq is mathematically irrelevant: softmax over the query axis is
    # invariant to per-key constant shifts.

    w_maps = {"wq": wq_h, "wkv": wkv_h, "bkv": bkv_h}

    in_maps = []
    chunks_by_batch = []
    for b_idx in range(B):
        xT = np.ascontiguousarray(x[b_idx].T)                  # [768, 4096]
        chunks = np.ascontiguousarray(
            xT.reshape(NC, P, NQC, QC).transpose(2, 1, 0, 3)
        ).astype(bf)                                           # [8,128,6,512]
        chunks_by_batch.append(chunks)
    for core in range(N_CORES):
        b_idx, h = divmod(core, 2)
        chunks = chunks_by_batch[b_idx]
        in_maps.append({
            "xtq": chunks,
            "xtk": np.ascontiguousarray(chunks[4 * h:4 * h + 4]),
            **w_maps,
        })

    res = run_bass_kernel_spmd(
        _get_nc(), in_maps, core_ids=list(range(N_CORES)), trace=False
    )

    out = np.empty((B, S, D), np.float32)
    for core in range(N_CORES):
        b_idx, h = divmod(core, 2)
        out[b_idx, h * SH:(h + 1) * SH, :] = res.results[core]["out"].T
    return out
